# revision 28
# baseline (speedup 1.0000x reference)
"""GraphSage 3-layer GNN on 8 Trainium2 NeuronCores.

Strategy: shard nodes (rows of A) across the 8 cores. A is a 0/1
adjacency (plus exact 1.0 self-loops) => exact in fp8_e4m3, so the
per-core A^T shard (12288x1536 = 18.9 MB fp8) is streamed from DRAM
ONCE and kept RESIDENT in SBUF (144 KB/partition). Layer 1 consumes
the chunks as they arrive (DMA-paced); layers 2-3 re-read A from SBUF
with zero HBM traffic. All three aggregation matmuls run fp8 with
perf_mode=DoubleRow (2 fp8 weights/PE cell, k-pairs contracted 256 at
a time) for ~1.5-2x PE throughput over bf16. The h/agg->z dense
matmuls + l2norm + tanh run in the transposed [feat, node] layout with
f32r moving operands (1 cyc/row); layers 2-3 are m-outer so each
m-tile's DVE/ACT tail overlaps the next m-tile's matmuls. An fp8
AllGather shares h between layers (PE kept HAM-warm through the
collective) and an AllReduce combines the global-sum-pool partials.

Numerics: stationary x/h quantized to fp8_e4m3 (RNE) only for the
mean-aggregation matmul; the concatenated self-features stay f32.
Measured on the harness input distribution this lands ~6e-3 relative
error vs the 2e-2 gate (A itself is exact in fp8).
"""

import os
import sys
import types

import numpy as np

# ---------------------------------------------------------------- ntff hook
# The image lacks antenv.axon_hooks; inject it so trace=True (profiling,
# enabled via BASS_TRACE=1 by test.py) can capture NTFF under axon.
def _install_ntff_hook():
    if "antenv.axon_hooks" in sys.modules:
        return
    try:
        import antenv
        mod = types.ModuleType("antenv.axon_hooks")
        _hook = [None]
        mod.set_axon_ntff_profile_hook = lambda h: _hook.__setitem__(0, h)
        mod.get_axon_ntff_profile_hook = lambda: _hook[0]
        sys.modules["antenv.axon_hooks"] = mod
        antenv.axon_hooks = mod
        from trn_agent_boot.trn_boot import _ntff_profile_via_ctypes
        so = "/opt/axon/libaxon_pjrt.so"
        if os.path.exists(so):
            mod.set_axon_ntff_profile_hook(_ntff_profile_via_ctypes(so))
    except Exception:
        pass


_install_ntff_hook()

import ml_dtypes  # noqa: E402
import concourse.bass as bass  # noqa: E402
import concourse.bacc as bacc  # noqa: E402
import concourse.tile as tile  # noqa: E402
import concourse.mybir as mybir  # noqa: E402
from concourse.bass_utils import run_bass_kernel_spmd  # noqa: E402

# ------------------------------------------------------------------ geometry
N = 12000          # real nodes
F = 128            # input feature dim
H = 32             # hidden dim
NC = 8             # cores
NP = 12288         # padded nodes  (= 96*128 = 8*1536)
SH = NP // NC      # 1536 rows per core
KC = NP // 128     # 96 contraction chunks
KP = KC // 2       # 48 DoubleRow k-pairs
MT = [(0, 512), (512, 512), (1024, 512)]   # m-tiles within the shard
NJ = SH // 128     # 12 transpose subtiles
TOL = 1e-6

MODE = os.environ.get("KMODE", "dr")       # "dr" (DoubleRow) | "flat"
WARM = int(os.environ.get("KWARM", "12"))  # warm-chain matmuls per collective

F32 = mybir.dt.float32
F32R = mybir.dt.float32r
BF16 = mybir.dt.bfloat16
FP8 = mybir.dt.float8e4
NP_FP8 = ml_dtypes.float8_e4m3
NP_BF16 = ml_dtypes.bfloat16
DR = (mybir.MatmulPerfMode.DoubleRow if MODE == "dr" else None)

LAST_EXEC_NS = None
_CACHE = {}


# ------------------------------------------------------------------- builder
def _build():
    nc = bacc.Bacc("TRN2", target_bir_lowering=False, debug=False,
                   num_devices=NC)

    at_d = nc.dram_tensor("at", [NP, SH], FP8, kind="ExternalInput")
    xs_d = nc.dram_tensor("xs", [NP, F], FP8, kind="ExternalInput")
    xt_d = nc.dram_tensor("xt", [F, SH], BF16, kind="ExternalInput")
    rc_d = nc.dram_tensor("rc", [F, SH], BF16, kind="ExternalInput")
    w1t_d = nc.dram_tensor("w1t", [F, H], BF16, kind="ExternalInput")
    w1b_d = nc.dram_tensor("w1b", [F, H], F32R, kind="ExternalInput")
    w2t_d = nc.dram_tensor("w2t", [H, H], F32R, kind="ExternalInput")
    w2b_d = nc.dram_tensor("w2b", [H, H], F32R, kind="ExternalInput")
    w3t_d = nc.dram_tensor("w3t", [H, H], F32R, kind="ExternalInput")
    w3b_d = nc.dram_tensor("w3b", [H, H], F32R, kind="ExternalInput")
    wf1_d = nc.dram_tensor("wf1", [H, 2 * H], F32, kind="ExternalInput")
    wf2_d = nc.dram_tensor("wf2", [2 * H, 1], F32, kind="ExternalInput")
    b1_d = nc.dram_tensor("b1", [H, 1], F32, kind="ExternalInput")
    b2_d = nc.dram_tensor("b2", [H, 1], F32, kind="ExternalInput")
    b3_d = nc.dram_tensor("b3", [H, 1], F32, kind="ExternalInput")
    bf1_d = nc.dram_tensor("bf1", [2 * H, 1], F32, kind="ExternalInput")
    bf2_d = nc.dram_tensor("bf2", [1, 1], F32, kind="ExternalInput")
    i32_d = nc.dram_tensor("i32", [32, 32], F32R, kind="ExternalInput")
    ones_d = nc.dram_tensor("ones", [H, H], F32R, kind="ExternalInput")
    out_d = nc.dram_tensor("out", [1, 1], F32, kind="ExternalOutput")

    # ag buffers hold h in hnat-major layout [(partition, subtile), feat]
    # so the post-gather stationary load is one DMA of large contiguous
    # segments (12 chunks x 32 B per core per partition) instead of 96
    # strided 32 B segments.
    ag_in = [nc.dram_tensor(f"ag_in{l}", [SH, H], FP8) for l in range(2)]
    ag_out = [nc.dram_tensor(f"ag_out{l}", [NP, H], FP8,
                             addr_space="Shared") for l in range(2)]
    ar_in = nc.dram_tensor("ar_in", [H, 1], F32)
    ar_out = nc.dram_tensor("ar_out", [H, 1], F32, addr_space="Shared")
    rg = [list(range(NC))]

    with tile.TileContext(nc) as tc:
        with (
            tc.tile_pool(name="const", bufs=1) as constp,
            tc.tile_pool(name="atres", bufs=1) as atresp,
            tc.tile_pool(name="xstat", bufs=1) as xstatp,
            tc.tile_pool(name="hstat", bufs=2) as hstatp,
            tc.tile_pool(name="hT", bufs=2) as hTp,
            tc.tile_pool(name="hnat", bufs=2) as hnatp,
            tc.tile_pool(name="ep", bufs=2) as ep,
            tc.tile_pool(name="agg_ps", bufs=3, space=bass.MemorySpace.PSUM) as agg_ps,
            tc.tile_pool(name="z_ps", bufs=2, space=bass.MemorySpace.PSUM) as z_ps,
            tc.tile_pool(name="bc_ps", bufs=1, space=bass.MemorySpace.PSUM) as bc_ps,
            tc.tile_pool(name="t_ps", bufs=2, space=bass.MemorySpace.PSUM) as t_ps,
        ):
            # x-stationary groups first on gpsimd: layer 1's matmuls are
            # paced by these + the at stream; xt/rc are only needed at the
            # layer-1 tails ~100us in, so they load last.
            xs = xstatp.tile([128, KC, F], FP8)
            xs_r = xs_d.ap().rearrange("(k p) f -> p k f", p=128)
            for g in range(0, KC, 8):
                nc.gpsimd.dma_start(xs[:, g:g + 8, :], xs_r[:, g:g + 8, :])

            # ---- constants
            def cload(dram, shape, dt=F32):
                t = constp.tile(shape, dt, tag=dram.name)
                nc.gpsimd.dma_start(t[:], dram[:, :])
                return t

            w1t = cload(w1t_d, [F, H], BF16)
            w1b = cload(w1b_d, [F, H], F32R)
            w2t = cload(w2t_d, [H, H], F32R)
            w2b = cload(w2b_d, [H, H], F32R)
            w3t = cload(w3t_d, [H, H], F32R)
            w3b = cload(w3b_d, [H, H], F32R)
            wf1 = cload(wf1_d, [H, 2 * H])
            wf2 = cload(wf2_d, [2 * H, 1])
            b1 = cload(b1_d, [H, 1])
            b2 = cload(b2_d, [H, 1])
            b3 = cload(b3_d, [H, 1])
            bf1 = cload(bf1_d, [2 * H, 1])
            bf2 = cload(bf2_d, [1, 1])
            i32 = cload(i32_d, [32, 32], F32R)
            ones_m = cload(ones_d, [H, H], F32R)
            xt = cload(xt_d, [F, SH], BF16)
            rc = cload(rc_d, [F, SH], BF16)

            # resident A^T shard, filled by layer 1's streaming loop
            at = atresp.tile([128, KC, SH], FP8)

            def big_matmul(li, pagg, h_stat, m0, mw, j):
                """one aggregation matmul: k-pair j into pagg[:, :mw]."""
                if MODE == "dr":
                    nc.tensor.matmul(
                        pagg[:, :mw], h_stat[:, 2 * j:2 * j + 2, :],
                        at[:, 2 * j:2 * j + 2, m0:m0 + mw],
                        start=(j == 0), stop=(j == KP - 1), perf_mode=DR)
                else:
                    for kk in (2 * j, 2 * j + 1):
                        nc.tensor.matmul(
                            pagg[:, :mw], h_stat[:, kk, :],
                            at[:, kk, m0:m0 + mw],
                            start=(kk == 0), stop=(kk == KC - 1))

            def tail(li, fl, mi, m0, mw, pagg, hT_in, wtop, wbot, b,
                     hTn, hnat, poff=0, red=None):
                """per-m-tile epilogue: mean-scale, dense, l2norm, tanh,
                plus transpose+export of the fp8 node-major h for layers
                with a following AllGather. poff: column offset into pagg;
                red: (tile, col) partial sum-pool target."""
                aggs = ep.tile([F, 512], F32R, tag="aggs")
                nc.vector.tensor_mul(
                    aggs[:fl, :mw], pagg[:fl, poff:poff + mw],
                    rc[:fl, m0:m0 + mw])
                pz = z_ps.tile([H, 512], F32, tag="pz")
                nc.tensor.matmul(pz[:, :mw], wtop[:, :], hT_in[:, m0:m0 + mw],
                                 start=True, stop=False)
                nc.tensor.matmul(pz[:, :mw], wbot[:, :], aggs[:fl, :mw],
                                 start=False, stop=True)
                zb = ep.tile([H, 512], F32, tag="zb")
                nc.vector.tensor_scalar_add(zb[:, :mw], pz[:, :mw], b[:])
                # row l2-norm over features (partition dim): sumsq via
                # ones-matmul broadcast back to H partitions.
                sq = ep.tile([H, 512], F32R, tag="sqzn")
                nc.vector.tensor_mul(sq[:, :mw], zb[:, :mw], zb[:, :mw])
                pbc = bc_ps.tile([H, 512], F32, tag="pbc")
                nc.tensor.matmul(pbc[:, :mw], ones_m[:, :], sq[:, :mw],
                                 start=True, stop=True)
                ssb = ep.tile([H, 512], F32, tag="ssb")
                nc.vector.tensor_scalar_max(ssb[:, :mw], pbc[:, :mw], 1e-12)
                srt = ep.tile([H, 512], F32, tag="sqzn")
                nc.scalar.sqrt(srt[:, :mw], ssb[:, :mw])
                rn = ep.tile([H, 512], F32, tag="rn")
                nc.vector.reciprocal_approx_fast(rn[:, :mw], srt[:, :mw])
                zn = ep.tile([H, 512], F32, tag="sqzn")
                nc.vector.tensor_mul(zn[:, :mw], zb[:, :mw], rn[:, :mw])
                nc.scalar.activation(hTn[:, m0:m0 + mw], zn[:, :mw],
                                     mybir.ActivationFunctionType.Tanh)
                if red is not None:
                    rtile, rcol = red
                    nc.vector.reduce_sum(rtile[:, rcol:rcol + 1],
                                         hTn[:, m0:m0 + mw],
                                         axis=mybir.AxisListType.X)
                if hnat is not None:
                    js, jc = m0 // 128, mw // 128
                    for j in range(js, js + jc):
                        pt = t_ps.tile([128, H], F32R, tag="pt")
                        nc.tensor.transpose(
                            pt[:, :], hTn[:, j * 128:(j + 1) * 128],
                            i32[:, :])
                        nc.vector.tensor_copy(hnat[:, j, :], pt[:, :])
                    agr = ag_in[li].ap().rearrange("(p j) f -> p j f", j=NJ)
                    nc.gpsimd.dma_start(
                        agr[:, js:js + jc, :], hnat[:, js:js + jc, :])

            def gather_and_load(li):
                """AllGather h, keep PE warm, load node-major h stationary."""
                nc.gpsimd.collective_compute(
                    "AllGather", mybir.AluOpType.bypass, replica_groups=rg,
                    ins=[ag_in[li].ap().opt()], outs=[ag_out[li].ap().opt()])
                # keep the PE HAM-warm through the collective stall
                if WARM > 0:
                    pw = bc_ps.tile([H, 512], F32, tag="pbc",
                                    name=f"warm{li}")
                    for dmy in range(WARM):
                        nc.tensor.matmul(pw[:, :], xs[:, 0, 0:H],
                                         xs[:, 0:4, :],
                                         start=(dmy == 0),
                                         stop=(dmy == WARM - 1))
                h_stat_n = hstatp.tile([128, KC, H], FP8, tag="hstat",
                                       name=f"hstat{li}")
                agor = ag_out[li].ap().rearrange(
                    "(c p j) f -> p c j f", p=128, j=NJ)
                dst = h_stat_n[:, :, :].rearrange("p (c j) f -> p c j f", c=NC)
                # split at the first core boundary so the next layer's first
                # k-pairs unblock as soon as 12 chunks have landed
                nc.sync.dma_start(dst[:, 0:1, :, :], agor[:, 0:1, :, :])
                nc.sync.dma_start(dst[:, 1:NC, :, :], agor[:, 1:NC, :, :])
                return h_stat_n

            # ----------------- layer 1: m-tile-major streaming of A into
            # SBUF. Column-block order lets m-tile 0/1 finish their k-loop
            # at 1/3 and 2/3 of the stream, hiding their tails (dense ops,
            # l2norm, transposes) under the remaining DMA; only m-tile 2's
            # tail sits on the critical path before the AllGather.
            hT1 = hTp.tile([H, SH], F32R, tag="hTn", name="hTn0")
            hnat0 = hnatp.tile([128, NJ, H], FP8, tag="hnat", name="hnat0")
            for mi, (m0, mw) in enumerate(MT):
                pagg = agg_ps.tile([F, 512], F32, tag="pagg",
                                   name=f"pagg0_{mi}")
                for j in range(KP):
                    nc.sync.dma_start(
                        at[:, 2 * j, m0:m0 + mw],
                        at_d[256 * j:256 * j + 128, m0:m0 + mw])
                    nc.sync.dma_start(
                        at[:, 2 * j + 1, m0:m0 + mw],
                        at_d[256 * j + 128:256 * j + 256, m0:m0 + mw])
                    big_matmul(0, pagg, xs, m0, mw, j)
                if mi == 2:
                    tail(0, F, mi, m0, 256, pagg, xt, w1t, w1b, b1,
                         hT1, hnat0, poff=0)
                    tail(0, F, mi, m0 + 256, 256, pagg, xt, w1t, w1b, b1,
                         hT1, hnat0, poff=256)
                else:
                    tail(0, F, mi, m0, mw, pagg, xt, w1t, w1b, b1,
                         hT1, hnat0)
            hs1 = gather_and_load(0)

            # ----------------- layers 2-3: m-outer, A resident in SBUF
            def layer23(li, hs, hT_in, wtop, wbot, b, hnat, pool=None):
                hTn = hTp.tile([H, SH], F32R, tag="hTn", name=f"hTn{li}")
                for mi, (m0, mw) in enumerate(MT):
                    pagg = agg_ps.tile([H, 512], F32, tag="pagg",
                                       name=f"pagg{li}_{mi}")
                    for j in range(KP):
                        big_matmul(li, pagg, hs, m0, mw, j)
                    if mi == 2:
                        # split the last tail so its serial DVE/ACT chain
                        # pipelines in two halves (it gates the next phase)
                        tail(li, H, mi, m0, 256, pagg, hT_in, wtop, wbot,
                             b, hTn, hnat, poff=0,
                             red=(pool, 2) if pool is not None else None)
                        tail(li, H, mi, m0 + 256, 256, pagg, hT_in, wtop,
                             wbot, b, hTn, hnat, poff=256,
                             red=(pool, 3) if pool is not None else None)
                    else:
                        tail(li, H, mi, m0, mw, pagg, hT_in, wtop, wbot, b,
                             hTn, hnat,
                             red=(pool, mi) if pool is not None else None)
                return hTn

            hnat1 = hnatp.tile([128, NJ, H], FP8, tag="hnat", name="hnat1")
            hT2 = layer23(1, hs1, hT1, w2t, w2b, b2, hnat1)
            hs2 = gather_and_load(1)
            pool4 = ep.tile([H, 4], F32, tag="pT")
            hT3 = layer23(2, hs2, hT2, w3t, w3b, b3, None, pool=pool4)

            # combine the per-m-tile pool partials (padded nodes are 0)
            pT = ep.tile([H, 1], F32, tag="pS")
            nc.vector.reduce_sum(pT[:, :], pool4[:, :],
                                 axis=mybir.AxisListType.X)
            nc.gpsimd.dma_start(ar_in[:, :], pT[:])
            nc.gpsimd.collective_compute(
                "AllReduce", mybir.AluOpType.add, replica_groups=rg,
                ins=[ar_in.ap().opt()], outs=[ar_out.ap().opt()])
            pS = ep.tile([H, 1], F32, tag="pS")
            nc.gpsimd.dma_start(pS[:], ar_out[:, :])

            # final MLP (redundant on every core)
            pq = z_ps.tile([2 * H, 1], F32, tag="pz")
            nc.tensor.matmul(pq[:, :], wf1[:, :], pS[:, :], start=True, stop=True)
            q = ep.tile([2 * H, 1], F32, tag="q")
            nc.scalar.activation(q[:, :], pq[:, :],
                                 mybir.ActivationFunctionType.Tanh,
                                 bias=bf1[:])
            po = z_ps.tile([1, 1], F32, tag="pz")
            nc.tensor.matmul(po[:, :], wf2[:, :], q[:, :], start=True, stop=True)
            ob = ep.tile([1, 1], F32, tag="ob")
            nc.vector.tensor_scalar_add(ob[:, :], po[:, :], bf2[:])
            nc.gpsimd.dma_start(out_d[:, :], ob[:])

    nc.compile()
    return nc


# ---------------------------------------------------------------- host prep
def _prep(inputs):
    x = np.asarray(inputs["x"], np.float32)
    a = np.asarray(inputs["a"], np.float32)
    diag = np.diagonal(a).copy()
    add = (np.abs(diag) < TOL).astype(np.float32)
    deg = a.sum(axis=1) + add          # row sums of a_hat
    recip = np.ones(NP, np.float32)
    recip[:N] = 1.0 / deg

    x_pad = np.zeros((NP, F), np.float32)
    x_pad[:N] = x
    xs = x_pad.astype(NP_FP8)

    w1 = np.asarray(inputs["W1"], np.float32)
    common = {
        "xs": xs,
        "w1t": w1[:F].astype(NP_BF16), "w1b": w1[F:].copy(),
        "w2t": np.asarray(inputs["W2"], np.float32)[:H].copy(),
        "w2b": np.asarray(inputs["W2"], np.float32)[H:].copy(),
        "w3t": np.asarray(inputs["W3"], np.float32)[:H].copy(),
        "w3b": np.asarray(inputs["W3"], np.float32)[H:].copy(),
        "wf1": np.asarray(inputs["Wf1"], np.float32),
        "wf2": np.asarray(inputs["Wf2"], np.float32),
        "b1": np.asarray(inputs["b1"], np.float32).reshape(H, 1),
        "b2": np.asarray(inputs["b2"], np.float32).reshape(H, 1),
        "b3": np.asarray(inputs["b3"], np.float32).reshape(H, 1),
        "bf1": np.asarray(inputs["bf1"], np.float32).reshape(2 * H, 1),
        "bf2": np.asarray(inputs["bf2"], np.float32).reshape(1, 1),
        "i32": np.eye(32, dtype=np.float32),
        "ones": np.ones((H, H), dtype=np.float32),
    }

    in_maps = []
    for c in range(NC):
        r0 = c * SH
        r1 = min((c + 1) * SH, N)
        nrow = max(r1 - r0, 0)
        at = np.zeros((NP, SH), NP_FP8)
        if nrow > 0:
            blk = a[r0:r1].T.astype(NP_FP8)         # [N(12000), nrow]
            at[:N, :nrow] = blk
            # self-loops on approximately-zero diagonal entries
            idx = np.arange(nrow)
            gi = r0 + idx
            sel = add[gi] > 0
            at[gi[sel], idx[sel]] = np.asarray(
                a[gi[sel], gi[sel]] + 1.0, NP_FP8)
        xt = np.zeros((F, SH), NP_BF16)
        if nrow > 0:
            xt[:, :nrow] = x[r0:r1].T.astype(NP_BF16)
        rcb = np.broadcast_to(recip[r0:r0 + SH].astype(NP_BF16),
                              (F, SH)).copy()
        m = dict(common)
        m.update({"at": at, "xt": xt, "rc": rcb})
        in_maps.append(m)
    return in_maps


# -------------------------------------------------------------------- kernel
def kernel(**inputs):
    global LAST_EXEC_NS
    if "nc" not in _CACHE:
        _CACHE["nc"] = _build()
    nc = _CACHE["nc"]
    in_maps = _prep(inputs)
    res = run_bass_kernel_spmd(nc, in_maps, core_ids=list(range(NC)))
    LAST_EXEC_NS = res.exec_time_ns
    return np.asarray(res.results[0]["out"], np.float32).reshape(1, 1)


# revision 30
# speedup vs baseline: 1.1836x; 1.1836x over previous
"""GraphSage 3-layer GNN on 8 Trainium2 NeuronCores.

Strategy: shard nodes (rows of A) across the 8 cores. A is a 0/1
adjacency (plus exact 1.0 self-loops) => exact in fp8_e4m3, so the
per-core A^T shard (12288x1536 = 18.9 MB fp8) is streamed from DRAM
ONCE and kept RESIDENT in SBUF (144 KB/partition). Layer 1 consumes
the chunks as they arrive (DMA-paced); layers 2-3 re-read A from SBUF
with zero HBM traffic. All three aggregation matmuls run fp8 with
perf_mode=DoubleRow (2 fp8 weights/PE cell, k-pairs contracted 256 at
a time) for ~1.5-2x PE throughput over bf16. The h/agg->z dense
matmuls + l2norm + tanh run in the transposed [feat, node] layout with
f32r moving operands (1 cyc/row); layers 2-3 are m-outer so each
m-tile's DVE/ACT tail overlaps the next m-tile's matmuls. An fp8
AllGather shares h between layers (PE kept HAM-warm through the
collective) and an AllReduce combines the global-sum-pool partials.

Numerics: stationary x/h quantized to fp8_e4m3 (RNE) only for the
mean-aggregation matmul; the concatenated self-features stay f32.
Measured on the harness input distribution this lands ~6e-3 relative
error vs the 2e-2 gate (A itself is exact in fp8).
"""

import os
import sys
import types

import numpy as np

# ---------------------------------------------------------------- ntff hook
# The image lacks antenv.axon_hooks; inject it so trace=True (profiling,
# enabled via BASS_TRACE=1 by test.py) can capture NTFF under axon.
def _install_ntff_hook():
    if "antenv.axon_hooks" in sys.modules:
        return
    try:
        import antenv
        mod = types.ModuleType("antenv.axon_hooks")
        _hook = [None]
        mod.set_axon_ntff_profile_hook = lambda h: _hook.__setitem__(0, h)
        mod.get_axon_ntff_profile_hook = lambda: _hook[0]
        sys.modules["antenv.axon_hooks"] = mod
        antenv.axon_hooks = mod
        from trn_agent_boot.trn_boot import _ntff_profile_via_ctypes
        so = "/opt/axon/libaxon_pjrt.so"
        if os.path.exists(so):
            mod.set_axon_ntff_profile_hook(_ntff_profile_via_ctypes(so))
    except Exception:
        pass


_install_ntff_hook()

import ml_dtypes  # noqa: E402
import concourse.bass as bass  # noqa: E402
import concourse.bacc as bacc  # noqa: E402
import concourse.tile as tile  # noqa: E402
import concourse.mybir as mybir  # noqa: E402
from concourse.bass_utils import run_bass_kernel_spmd  # noqa: E402

# ------------------------------------------------------------------ geometry
N = 12000          # real nodes
F = 128            # input feature dim
H = 32             # hidden dim
NC = 8             # cores
NP = 12288         # padded nodes  (= 96*128 = 8*1536)
SH = NP // NC      # 1536 rows per core
KC = NP // 128     # 96 contraction chunks
KP = KC // 2       # 48 DoubleRow k-pairs
MT = [(0, 512), (512, 512), (1024, 512)]   # m-tiles within the shard
NJ = SH // 128     # 12 transpose subtiles
TOL = 1e-6

MODE = os.environ.get("KMODE", "dr")       # "dr" (DoubleRow) | "flat"
WARM = int(os.environ.get("KWARM", "12"))  # warm-chain matmuls per collective

F32 = mybir.dt.float32
F32R = mybir.dt.float32r
BF16 = mybir.dt.bfloat16
FP8 = mybir.dt.float8e4
NP_FP8 = ml_dtypes.float8_e4m3
NP_BF16 = ml_dtypes.bfloat16
DR = (mybir.MatmulPerfMode.DoubleRow if MODE == "dr" else None)

LAST_EXEC_NS = None
_CACHE = {}


# ------------------------------------------------------------------- builder
def _build():
    nc = bacc.Bacc("TRN2", target_bir_lowering=False, debug=False,
                   num_devices=NC)

    at_d = nc.dram_tensor("at", [NP, SH], FP8, kind="ExternalInput")
    xs_d = nc.dram_tensor("xs", [NP, F], FP8, kind="ExternalInput")
    xt_d = nc.dram_tensor("xt", [F, SH], BF16, kind="ExternalInput")
    rc_d = nc.dram_tensor("rc", [F, SH], BF16, kind="ExternalInput")
    w1t_d = nc.dram_tensor("w1t", [F, H], BF16, kind="ExternalInput")
    w1b_d = nc.dram_tensor("w1b", [F, H], F32R, kind="ExternalInput")
    w2t_d = nc.dram_tensor("w2t", [H, H], F32R, kind="ExternalInput")
    w2b_d = nc.dram_tensor("w2b", [H, H], F32R, kind="ExternalInput")
    w3t_d = nc.dram_tensor("w3t", [H, H], F32R, kind="ExternalInput")
    w3b_d = nc.dram_tensor("w3b", [H, H], F32R, kind="ExternalInput")
    wf1_d = nc.dram_tensor("wf1", [H, 2 * H], F32, kind="ExternalInput")
    wf2_d = nc.dram_tensor("wf2", [2 * H, 1], F32, kind="ExternalInput")
    b1_d = nc.dram_tensor("b1", [H, 1], F32, kind="ExternalInput")
    b2_d = nc.dram_tensor("b2", [H, 1], F32, kind="ExternalInput")
    b3_d = nc.dram_tensor("b3", [H, 1], F32, kind="ExternalInput")
    bf1_d = nc.dram_tensor("bf1", [2 * H, 1], F32, kind="ExternalInput")
    bf2_d = nc.dram_tensor("bf2", [1, 1], F32, kind="ExternalInput")
    i32_d = nc.dram_tensor("i32", [32, 32], F32R, kind="ExternalInput")
    ones_d = nc.dram_tensor("ones", [H, H], F32R, kind="ExternalInput")
    out_d = nc.dram_tensor("out", [1, 1], F32, kind="ExternalOutput")

    # ag buffers hold h in hnat-major layout [(partition, subtile), feat]
    # so the post-gather stationary load is one DMA of large contiguous
    # segments (12 chunks x 32 B per core per partition) instead of 96
    # strided 32 B segments.
    ag_in = [nc.dram_tensor(f"ag_in{l}", [SH, H], FP8) for l in range(2)]
    ag_out = [nc.dram_tensor(f"ag_out{l}", [NP, H], FP8,
                             addr_space="Shared") for l in range(2)]
    ar_in = nc.dram_tensor("ar_in", [H, 1], F32)
    ar_out = nc.dram_tensor("ar_out", [H, 1], F32, addr_space="Shared")
    rg = [list(range(NC))]

    with tile.TileContext(nc) as tc:
        with (
            tc.tile_pool(name="const", bufs=1) as constp,
            tc.tile_pool(name="atres", bufs=1) as atresp,
            tc.tile_pool(name="xstat", bufs=1) as xstatp,
            tc.tile_pool(name="hstat", bufs=2) as hstatp,
            tc.tile_pool(name="hT", bufs=2) as hTp,
            tc.tile_pool(name="hnat", bufs=2) as hnatp,
            tc.tile_pool(name="ep", bufs=2) as ep,
            tc.tile_pool(name="agg_ps", bufs=3, space=bass.MemorySpace.PSUM) as agg_ps,
            tc.tile_pool(name="z_ps", bufs=2, space=bass.MemorySpace.PSUM) as z_ps,
            tc.tile_pool(name="bc_ps", bufs=1, space=bass.MemorySpace.PSUM) as bc_ps,
            tc.tile_pool(name="t_ps", bufs=2, space=bass.MemorySpace.PSUM) as t_ps,
        ):
            # x-stationary groups first on gpsimd: layer 1's matmuls are
            # paced by these + the at stream; xt/rc are only needed at the
            # layer-1 tails ~100us in, so they load last.
            xs = xstatp.tile([128, KC, F], FP8)
            xs_r = xs_d.ap().rearrange("(k p) f -> p k f", p=128)
            for g in range(0, KC, 8):
                nc.gpsimd.dma_start(xs[:, g:g + 8, :], xs_r[:, g:g + 8, :])

            # ---- constants
            def cload(dram, shape, dt=F32):
                t = constp.tile(shape, dt, tag=dram.name)
                nc.gpsimd.dma_start(t[:], dram[:, :])
                return t

            w1t = cload(w1t_d, [F, H], BF16)
            w1b = cload(w1b_d, [F, H], F32R)
            w2t = cload(w2t_d, [H, H], F32R)
            w2b = cload(w2b_d, [H, H], F32R)
            w3t = cload(w3t_d, [H, H], F32R)
            w3b = cload(w3b_d, [H, H], F32R)
            wf1 = cload(wf1_d, [H, 2 * H])
            wf2 = cload(wf2_d, [2 * H, 1])
            b1 = cload(b1_d, [H, 1])
            b2 = cload(b2_d, [H, 1])
            b3 = cload(b3_d, [H, 1])
            bf1 = cload(bf1_d, [2 * H, 1])
            bf2 = cload(bf2_d, [1, 1])
            i32 = cload(i32_d, [32, 32], F32R)
            ones_m = cload(ones_d, [H, H], F32R)
            xt = cload(xt_d, [F, SH], BF16)
            rc = cload(rc_d, [F, SH], BF16)

            # resident A^T shard, filled by layer 1's streaming loop
            at = atresp.tile([128, KC, SH], FP8)

            def big_matmul(li, pagg, h_stat, m0, mw, j):
                """one aggregation matmul: k-pair j into pagg[:, :mw]."""
                if MODE == "dr":
                    nc.tensor.matmul(
                        pagg[:, :mw], h_stat[:, 2 * j:2 * j + 2, :],
                        at[:, 2 * j:2 * j + 2, m0:m0 + mw],
                        start=(j == 0), stop=(j == KP - 1), perf_mode=DR)
                else:
                    for kk in (2 * j, 2 * j + 1):
                        nc.tensor.matmul(
                            pagg[:, :mw], h_stat[:, kk, :],
                            at[:, kk, m0:m0 + mw],
                            start=(kk == 0), stop=(kk == KC - 1))

            def tail(li, fl, mi, m0, mw, pagg, hT_in, wtop, wbot, b,
                     hTn, hnat, poff=0, red=None):
                """per-m-tile epilogue: mean-scale, dense, l2norm, tanh,
                plus transpose+export of the fp8 node-major h for layers
                with a following AllGather. poff: column offset into pagg;
                red: (tile, col) partial sum-pool target."""
                aggs = ep.tile([F, 512], F32R, tag="aggs")
                nc.vector.tensor_mul(
                    aggs[:fl, :mw], pagg[:fl, poff:poff + mw],
                    rc[:fl, m0:m0 + mw])
                pz = z_ps.tile([H, 512], F32, tag="pz")
                nc.tensor.matmul(pz[:, :mw], wtop[:, :], hT_in[:, m0:m0 + mw],
                                 start=True, stop=False)
                nc.tensor.matmul(pz[:, :mw], wbot[:, :], aggs[:fl, :mw],
                                 start=False, stop=True)
                zb = ep.tile([H, 512], F32, tag="zb")
                nc.vector.tensor_scalar_add(zb[:, :mw], pz[:, :mw], b[:])
                # row l2-norm over features (partition dim): sumsq via
                # ones-matmul broadcast back to H partitions.
                sq = ep.tile([H, 512], F32R, tag="sqzn")
                nc.vector.tensor_mul(sq[:, :mw], zb[:, :mw], zb[:, :mw])
                pbc = bc_ps.tile([H, 512], F32, tag="pbc")
                nc.tensor.matmul(pbc[:, :mw], ones_m[:, :], sq[:, :mw],
                                 start=True, stop=True)
                ssb = ep.tile([H, 512], F32, tag="ssb")
                nc.vector.tensor_scalar_max(ssb[:, :mw], pbc[:, :mw], 1e-12)
                srt = ep.tile([H, 512], F32, tag="sqzn")
                nc.scalar.sqrt(srt[:, :mw], ssb[:, :mw])
                rn = ep.tile([H, 512], F32, tag="rn")
                nc.vector.reciprocal_approx_fast(rn[:, :mw], srt[:, :mw])
                zn = ep.tile([H, 512], F32, tag="sqzn")
                nc.vector.tensor_mul(zn[:, :mw], zb[:, :mw], rn[:, :mw])
                nc.scalar.activation(hTn[:, m0:m0 + mw], zn[:, :mw],
                                     mybir.ActivationFunctionType.Tanh)
                if red is not None:
                    rtile, rcol = red
                    nc.vector.reduce_sum(rtile[:, rcol:rcol + 1],
                                         hTn[:, m0:m0 + mw],
                                         axis=mybir.AxisListType.X)
                if hnat is not None:
                    js, jc = m0 // 128, mw // 128
                    for j in range(js, js + jc):
                        pt = t_ps.tile([128, H], F32R, tag="pt")
                        nc.tensor.transpose(
                            pt[:, :], hTn[:, j * 128:(j + 1) * 128],
                            i32[:, :])
                        nc.vector.tensor_copy(hnat[:, j, :], pt[:, :])
                    agr = ag_in[li].ap().rearrange("(p j) f -> p j f", j=NJ)
                    nc.gpsimd.dma_start(
                        agr[:, js:js + jc, :], hnat[:, js:js + jc, :])

            def gather_and_load(li):
                """AllGather h, keep PE warm, load node-major h stationary."""
                nc.gpsimd.collective_compute(
                    "AllGather", mybir.AluOpType.bypass, replica_groups=rg,
                    ins=[ag_in[li].ap().opt()], outs=[ag_out[li].ap().opt()])
                # keep the PE HAM-warm through the collective stall
                if WARM > 0:
                    pw = bc_ps.tile([H, 512], F32, tag="pbc",
                                    name=f"warm{li}")
                    for dmy in range(WARM):
                        nc.tensor.matmul(pw[:, :], xs[:, 0, 0:H],
                                         xs[:, 0:4, :],
                                         start=(dmy == 0),
                                         stop=(dmy == WARM - 1))
                h_stat_n = hstatp.tile([128, KC, H], FP8, tag="hstat",
                                       name=f"hstat{li}")
                agor = ag_out[li].ap().rearrange(
                    "(c p j) f -> p c j f", p=128, j=NJ)
                dst = h_stat_n[:, :, :].rearrange("p (c j) f -> p c j f", c=NC)
                # split at the first core boundary so the next layer's first
                # k-pairs unblock as soon as 12 chunks have landed
                nc.sync.dma_start(dst[:, 0:1, :, :], agor[:, 0:1, :, :])
                nc.sync.dma_start(dst[:, 1:NC, :, :], agor[:, 1:NC, :, :])
                return h_stat_n

            # ----------------- layer 1: k-outer, DMA-streams A into SBUF
            # (full-row chunks keep the DMA descriptors contiguous 192 KB).
            # All three tails land after the stream; split each into halves
            # so the DVE/ACT chains pipeline across engines.
            hT1 = hTp.tile([H, SH], F32R, tag="hTn", name="hTn0")
            hnat0 = hnatp.tile([128, NJ, H], FP8, tag="hnat", name="hnat0")
            paggs = [agg_ps.tile([F, 512], F32, tag="pagg",
                                 name=f"pagg0_{mi}") for mi in range(3)]
            for j in range(KP):
                nc.sync.dma_start(
                    at[:, 2 * j, :], at_d[256 * j:256 * j + 128, :])
                nc.sync.dma_start(
                    at[:, 2 * j + 1, :], at_d[256 * j + 128:256 * j + 256, :])
                for mi, (m0, mw) in enumerate(MT):
                    big_matmul(0, paggs[mi], xs, m0, mw, j)
            for mi, (m0, mw) in enumerate(MT):
                for half in range(2):
                    tail(0, F, mi, m0 + 256 * half, 256, paggs[mi],
                         xt, w1t, w1b, b1, hT1, hnat0, poff=256 * half)
            hs1 = gather_and_load(0)

            # ----------------- layers 2-3: m-outer, A resident in SBUF
            def layer23(li, hs, hT_in, wtop, wbot, b, hnat, pool=None):
                hTn = hTp.tile([H, SH], F32R, tag="hTn", name=f"hTn{li}")
                for mi, (m0, mw) in enumerate(MT):
                    pagg = agg_ps.tile([H, 512], F32, tag="pagg",
                                       name=f"pagg{li}_{mi}")
                    for j in range(KP):
                        big_matmul(li, pagg, hs, m0, mw, j)
                    if mi == 2:
                        # split the last tail so its serial DVE/ACT chain
                        # pipelines in two halves (it gates the next phase)
                        tail(li, H, mi, m0, 256, pagg, hT_in, wtop, wbot,
                             b, hTn, hnat, poff=0,
                             red=(pool, 2) if pool is not None else None)
                        tail(li, H, mi, m0 + 256, 256, pagg, hT_in, wtop,
                             wbot, b, hTn, hnat, poff=256,
                             red=(pool, 3) if pool is not None else None)
                    else:
                        tail(li, H, mi, m0, mw, pagg, hT_in, wtop, wbot, b,
                             hTn, hnat,
                             red=(pool, mi) if pool is not None else None)
                return hTn

            hnat1 = hnatp.tile([128, NJ, H], FP8, tag="hnat", name="hnat1")
            hT2 = layer23(1, hs1, hT1, w2t, w2b, b2, hnat1)
            hs2 = gather_and_load(1)
            pool4 = ep.tile([H, 4], F32, tag="pT")
            hT3 = layer23(2, hs2, hT2, w3t, w3b, b3, None, pool=pool4)

            # combine the per-m-tile pool partials (padded nodes are 0).
            # ar_in's write goes on the vector queue so the gpsimd queue
            # arms the AllReduce early — its ~10us mesh setup then overlaps
            # layer 3's last matmuls instead of sitting on the tail.
            pT = ep.tile([H, 1], F32, tag="pS")
            nc.vector.reduce_sum(pT[:, :], pool4[:, :],
                                 axis=mybir.AxisListType.X)
            nc.vector.dma_start(ar_in[:, :], pT[:])
            nc.gpsimd.collective_compute(
                "AllReduce", mybir.AluOpType.add, replica_groups=rg,
                ins=[ar_in.ap().opt()], outs=[ar_out.ap().opt()])
            pS = ep.tile([H, 1], F32, tag="pS")
            nc.gpsimd.dma_start(pS[:], ar_out[:, :])

            # final MLP (redundant on every core)
            pq = z_ps.tile([2 * H, 1], F32, tag="pz")
            nc.tensor.matmul(pq[:, :], wf1[:, :], pS[:, :], start=True, stop=True)
            q = ep.tile([2 * H, 1], F32, tag="q")
            nc.scalar.activation(q[:, :], pq[:, :],
                                 mybir.ActivationFunctionType.Tanh,
                                 bias=bf1[:])
            po = z_ps.tile([1, 1], F32, tag="pz")
            nc.tensor.matmul(po[:, :], wf2[:, :], q[:, :], start=True, stop=True)
            ob = ep.tile([1, 1], F32, tag="ob")
            nc.vector.tensor_scalar_add(ob[:, :], po[:, :], bf2[:])
            nc.gpsimd.dma_start(out_d[:, :], ob[:])

    nc.compile()
    return nc


# ---------------------------------------------------------------- host prep
def _prep(inputs):
    x = np.asarray(inputs["x"], np.float32)
    a = np.asarray(inputs["a"], np.float32)
    diag = np.diagonal(a).copy()
    add = (np.abs(diag) < TOL).astype(np.float32)
    deg = a.sum(axis=1) + add          # row sums of a_hat
    recip = np.ones(NP, np.float32)
    recip[:N] = 1.0 / deg

    x_pad = np.zeros((NP, F), np.float32)
    x_pad[:N] = x
    xs = x_pad.astype(NP_FP8)

    w1 = np.asarray(inputs["W1"], np.float32)
    common = {
        "xs": xs,
        "w1t": w1[:F].astype(NP_BF16), "w1b": w1[F:].copy(),
        "w2t": np.asarray(inputs["W2"], np.float32)[:H].copy(),
        "w2b": np.asarray(inputs["W2"], np.float32)[H:].copy(),
        "w3t": np.asarray(inputs["W3"], np.float32)[:H].copy(),
        "w3b": np.asarray(inputs["W3"], np.float32)[H:].copy(),
        "wf1": np.asarray(inputs["Wf1"], np.float32),
        "wf2": np.asarray(inputs["Wf2"], np.float32),
        "b1": np.asarray(inputs["b1"], np.float32).reshape(H, 1),
        "b2": np.asarray(inputs["b2"], np.float32).reshape(H, 1),
        "b3": np.asarray(inputs["b3"], np.float32).reshape(H, 1),
        "bf1": np.asarray(inputs["bf1"], np.float32).reshape(2 * H, 1),
        "bf2": np.asarray(inputs["bf2"], np.float32).reshape(1, 1),
        "i32": np.eye(32, dtype=np.float32),
        "ones": np.ones((H, H), dtype=np.float32),
    }

    in_maps = []
    for c in range(NC):
        r0 = c * SH
        r1 = min((c + 1) * SH, N)
        nrow = max(r1 - r0, 0)
        at = np.zeros((NP, SH), NP_FP8)
        if nrow > 0:
            blk = a[r0:r1].T.astype(NP_FP8)         # [N(12000), nrow]
            at[:N, :nrow] = blk
            # self-loops on approximately-zero diagonal entries
            idx = np.arange(nrow)
            gi = r0 + idx
            sel = add[gi] > 0
            at[gi[sel], idx[sel]] = np.asarray(
                a[gi[sel], gi[sel]] + 1.0, NP_FP8)
        xt = np.zeros((F, SH), NP_BF16)
        if nrow > 0:
            xt[:, :nrow] = x[r0:r1].T.astype(NP_BF16)
        rcb = np.broadcast_to(recip[r0:r0 + SH].astype(NP_BF16),
                              (F, SH)).copy()
        m = dict(common)
        m.update({"at": at, "xt": xt, "rc": rcb})
        in_maps.append(m)
    return in_maps


# -------------------------------------------------------------------- kernel
def kernel(**inputs):
    global LAST_EXEC_NS
    if "nc" not in _CACHE:
        _CACHE["nc"] = _build()
    nc = _CACHE["nc"]
    in_maps = _prep(inputs)
    res = run_bass_kernel_spmd(nc, in_maps, core_ids=list(range(NC)))
    LAST_EXEC_NS = res.exec_time_ns
    return np.asarray(res.results[0]["out"], np.float32).reshape(1, 1)
